# revision 11
# baseline (speedup 1.0000x reference)
"""Causal depthwise conv1d (K=4) + SiLU, sharded over 8 NeuronCores.

Full shapes: x [4, 8192, 2048] f32, weight [2048, 4] f32 -> y [4, 8192, 2048] f32.

Strategy: tensor-parallel over the hidden/channel dim (fully channel
independent, no halo exchange). Each core gets 256 channels, reorganized
host-side to channel-major [B*256, S] so the conv runs along the free dim with
channels on SBUF partitions.

Raw bass (no Tile framework): the installed walrus codegen only accepts one
sync wait per compute instruction, so all synchronization is explicit
wait_ge sequencer instructions. Pipeline per token tile:
  SP:  input DMA + (SiLU-gated) output DMA
  ACT: tap0 as scaled copy, SiLU
  DVE: taps 1-3 as fused scalar_tensor_tensor MACs

DMA completion increments from concurrent transfers interleave on a shared
semaphore, so each buffer slot gets its own DMA semaphore; the slot-reuse
(WAR) waits guarantee a slot's previous DMA fully completed before the next
one on the same semaphore is issued.
"""

import contextlib

import numpy as np

B, S, H, K = 4, 8192, 2048, 4
N_CORES = 8
HC = H // N_CORES          # 256 channels per core
ROWS = B * HC              # 1024 rows per core, row r = b*HC + c
NU = ROWS // 128           # 8 partition units
T = 2048                   # token tile
NT = S // T
NTILES = NU * NT           # 32
NB = 4                     # buffers per tile kind

_last_results = None       # test harness introspection (exec_time_ns etc.)
_ACT_FUNC = "Silu"         # sim override hook (CoreSim lacks Silu)


def _build_program():
    from concourse import bass, mybir

    f32 = mybir.dt.float32
    mult = mybir.AluOpType.mult
    add = mybir.AluOpType.add
    AF = mybir.ActivationFunctionType

    nc = bass.Bass()
    x_d = nc.declare_dram_parameter("x", [ROWS, S], f32, isOutput=False)
    # weight cols: NU*K tap weights + one zero column (bias for Silu)
    w_d = nc.declare_dram_parameter("w", [128, NU * K + 1], f32, isOutput=False)
    y_d = nc.declare_dram_parameter("y", [ROWS, S], f32, isOutput=True)

    with contextlib.ExitStack() as st:
        wt = st.enter_context(nc.sbuf_tensor("wt", [128, NU * K + 1], f32))
        xts = [
            st.enter_context(nc.sbuf_tensor(f"xt{i}", [128, T + 3], f32))
            for i in range(NB)
        ]
        ms = [
            st.enter_context(nc.sbuf_tensor(f"m{i}", [128, T], f32))
            for i in range(NB)
        ]
        yts = [
            st.enter_context(nc.sbuf_tensor(f"yt{i}", [128, T], f32))
            for i in range(NB)
        ]
        zb = wt[:, NU * K : NU * K + 1]           # zeros column (Silu bias)

        def w_ap(k, i):
            u = k // NT
            return wt[:, u * K + i : u * K + i + 1]

        def x_rows(k):
            r0 = (k // NT) * 128
            return r0, r0 + 128

        with (
            nc.Block() as block,
            nc.semaphore("wsem") as wsem,
            nc.semaphore("act") as act,
            nc.semaphore("dve") as dve,
            contextlib.ExitStack() as sems,
        ):
            din = [
                sems.enter_context(nc.semaphore(f"din{i}")) for i in range(NB)
            ]
            dout = [
                sems.enter_context(nc.semaphore(f"dout{i}")) for i in range(NB)
            ]

            @block.sync
            def _(sync):
                def store(j):
                    # output DMA for tile j, gated on its SiLU completion.
                    # silu_j is ACT's (copies-so-far + silus-so-far)'th inc:
                    sync.wait_ge(act, min(j + 2, NTILES) + j + 1)
                    r0, r1 = x_rows(j)
                    t0 = (j % NT) * T
                    sync.dma_start(
                        out=y_d[r0:r1, t0 : t0 + T], in_=yts[j % NB][:, :]
                    ).then_inc(dout[j % NB], 16)

                sync.dma_start(out=wt[:, :], in_=w_d[:, :]).then_inc(wsem, 16)
                for k in range(NTILES):
                    r0, r1 = x_rows(k)
                    t = k % NT
                    t0 = t * T
                    if k >= NB:
                        # xt/m/yt slot free once DVE finished tile k-NB;
                        # also guarantees din[k%NB]'s previous DMA completed
                        sync.wait_ge(dve, k - NB + 1)
                    if t == 0:
                        sync.dma_start(
                            out=xts[k % NB][:, 3 : T + 3],
                            in_=x_d[r0:r1, 0:T],
                        ).then_inc(din[k % NB], 16)
                    else:
                        sync.dma_start(
                            out=xts[k % NB][:, :],
                            in_=x_d[r0:r1, t0 - 3 : t0 + T],
                        ).then_inc(din[k % NB], 16)
                    if k >= 1:
                        store(k - 1)
                store(NTILES - 1)
                for i in range(NB):
                    n_stores = len([k for k in range(NTILES) if k % NB == i])
                    sync.wait_ge(dout[i], 16 * n_stores)

            @block.scalar
            def _(scalar):
                def finish(j):
                    # SiLU for tile j (m[j] complete once dve >= j+1)
                    scalar.wait_ge(dve, j + 1)
                    if j >= NB:
                        # yt slot's previous store (tile j-NB) must be done
                        scalar.wait_ge(dout[j % NB], 16 * (j // NB))
                    func = getattr(AF, _ACT_FUNC)
                    scalar.activation(
                        out=yts[j % NB][:, :], in_=ms[j % NB][:, :],
                        func=func,
                        bias=0.0 if func == AF.Copy else zb,
                        scale=1.0,
                    ).then_inc(act)

                scalar.wait_ge(wsem, 16)
                for k in range(NTILES):
                    scalar.wait_ge(din[k % NB], 16 * (k // NB + 1))
                    xt = xts[k % NB]
                    m = ms[k % NB]
                    if k % NT == 0:
                        # causal start: begin with tap 3 over the full tile
                        scalar.activation(
                            out=m[:, :], in_=xt[:, 3 : 3 + T],
                            func=AF.Copy, bias=0.0, scale=w_ap(k, 3),
                        ).then_inc(act)
                    else:
                        scalar.activation(
                            out=m[:, :], in_=xt[:, 0:T],
                            func=AF.Copy, bias=0.0, scale=w_ap(k, 0),
                        ).then_inc(act)
                    if k >= 1:
                        finish(k - 1)
                finish(NTILES - 1)

            @block.vector
            def _(vector):
                def stt(out, in0, scalar_ap, in1):
                    return vector.scalar_tensor_tensor(
                        out=out, in0=in0, scalar=scalar_ap, in1=in1,
                        op0=mult, op1=add,
                    )

                for k in range(NTILES):
                    vector.wait_ge(act, 1 if k == 0 else 2 * k)
                    xt = xts[k % NB]
                    m = ms[k % NB]
                    if k % NT == 0:
                        # column j only sees taps with j-3+i >= 0
                        stt(m[:, 1:T], xt[:, 3 : T + 2], w_ap(k, 2), m[:, 1:T])
                        stt(m[:, 2:T], xt[:, 3 : T + 1], w_ap(k, 1), m[:, 2:T])
                        stt(m[:, 3:T], xt[:, 3:T], w_ap(k, 0), m[:, 3:T]).then_inc(dve)
                    else:
                        stt(m[:, :], xt[:, 1 : 1 + T], w_ap(k, 1), m[:, :])
                        stt(m[:, :], xt[:, 2 : 2 + T], w_ap(k, 2), m[:, :])
                        stt(m[:, :], xt[:, 3 : 3 + T], w_ap(k, 3), m[:, :]).then_inc(dve)

    return nc


def kernel(x, weight):
    global _last_results
    from concourse.bass_utils import run_bass_kernel_spmd

    x = np.asarray(x, dtype=np.float32)
    weight = np.asarray(weight, dtype=np.float32)

    nc = _build_program()

    in_maps = []
    for core in range(N_CORES):
        sl = slice(core * HC, (core + 1) * HC)
        # [B, S, HC] -> [B, HC, S] -> [ROWS, S], row r = b*HC + c
        xs = np.ascontiguousarray(x[:, :, sl].transpose(0, 2, 1)).reshape(ROWS, S)
        ws = weight[sl, :]  # (HC, K)
        w_host = np.zeros((128, NU * K + 1), np.float32)
        for u in range(NU):
            blk = u % (HC // 128)
            w_host[:, u * K : (u + 1) * K] = ws[blk * 128 : (blk + 1) * 128, :]
        in_maps.append({"x": xs, "w": w_host})

    res = run_bass_kernel_spmd(nc, in_maps, list(range(N_CORES)))
    _last_results = res

    out = np.empty((B, S, H), np.float32)
    for core in range(N_CORES):
        sl = slice(core * HC, (core + 1) * HC)
        yc = res.results[core]["y"].reshape(B, HC, S)
        out[:, :, sl] = yc.transpose(0, 2, 1)
    return out


# revision 15
# speedup vs baseline: 1.6389x; 1.6389x over previous
"""Causal depthwise conv1d (K=4) + SiLU, sharded over 8 NeuronCores.

Full shapes: x [4, 8192, 2048] f32, weight [2048, 4] f32 -> y [4, 8192, 2048] f32.

Strategy: tensor-parallel over the hidden/channel dim (fully channel
independent, no halo exchange). Each core gets 256 channels, reorganized
host-side to channel-major [B*256, S] so the conv runs along the free dim with
channels on SBUF partitions.

Raw bass (no Tile framework): the installed walrus codegen only accepts one
sync wait per compute instruction, so all synchronization is explicit
wait_ge sequencer instructions. Pipeline per token tile:
  SP:  input DMA + (SiLU-gated) output DMA
  ACT: tap0 as scaled copy, SiLU
  DVE: taps 1-3 as fused scalar_tensor_tensor MACs

DMA completion increments from concurrent transfers interleave on a shared
semaphore, so each buffer slot gets its own DMA semaphore; the slot-reuse
(WAR) waits guarantee a slot's previous DMA fully completed before the next
one on the same semaphore is issued.
"""

import contextlib

import numpy as np

B, S, H, K = 4, 8192, 2048, 4
N_CORES = 8
HC = H // N_CORES          # 256 channels per core
ROWS = B * HC              # 1024 rows per core, row r = b*HC + c
NU = ROWS // 128           # 8 partition units
T = 2048                   # token tile
NT = S // T
NTILES = NU * NT           # 32
NB = 4                     # buffers per tile kind

_last_results = None       # test harness introspection (exec_time_ns etc.)
_ACT_FUNC = "Silu"         # sim override hook (CoreSim lacks Silu)


def _build_program():
    from concourse import bass, mybir

    f32 = mybir.dt.float32
    mult = mybir.AluOpType.mult
    add = mybir.AluOpType.add
    AF = mybir.ActivationFunctionType

    nc = bass.Bass()
    x_d = nc.declare_dram_parameter("x", [ROWS, S], f32, isOutput=False)
    # weight cols: NU*K tap weights + one zero column (bias for Silu)
    w_d = nc.declare_dram_parameter("w", [128, NU * K + 1], f32, isOutput=False)
    y_d = nc.declare_dram_parameter("y", [ROWS, S], f32, isOutput=True)

    with contextlib.ExitStack() as st:
        wt = st.enter_context(nc.sbuf_tensor("wt", [128, NU * K + 1], f32))
        xts = [
            st.enter_context(nc.sbuf_tensor(f"xt{i}", [128, T + 3], f32))
            for i in range(NB)
        ]
        ms = [
            st.enter_context(nc.sbuf_tensor(f"m{i}", [128, T], f32))
            for i in range(NB)
        ]
        yts = [
            st.enter_context(nc.sbuf_tensor(f"yt{i}", [128, T], f32))
            for i in range(NB)
        ]
        zb = wt[:, NU * K : NU * K + 1]           # zeros column (Silu bias)

        def w_ap(k, i):
            u = k // NT
            return wt[:, u * K + i : u * K + i + 1]

        def x_rows(k):
            r0 = (k // NT) * 128
            return r0, r0 + 128

        with (
            nc.Block() as block,
            nc.semaphore("wsem") as wsem,
            nc.semaphore("act") as act,
            nc.semaphore("dve") as dve,
            contextlib.ExitStack() as sems,
        ):
            din = [
                sems.enter_context(nc.semaphore(f"din{i}")) for i in range(NB)
            ]
            dout = [
                sems.enter_context(nc.semaphore(f"dout{i}")) for i in range(NB)
            ]

            @block.sync
            def _(sync):
                sync.dma_start(out=wt[:, :], in_=w_d[:, :]).then_inc(wsem, 16)
                for k in range(NTILES):
                    r0, r1 = x_rows(k)
                    t = k % NT
                    t0 = t * T
                    if k >= NB:
                        # xt/m/yt slot free once DVE finished tile k-NB;
                        # also guarantees din[k%NB]'s previous DMA completed
                        sync.wait_ge(dve, k - NB + 1)
                    if t == 0:
                        sync.dma_start(
                            out=xts[k % NB][:, 3 : T + 3],
                            in_=x_d[r0:r1, 0:T],
                        ).then_inc(din[k % NB], 16)
                    else:
                        sync.dma_start(
                            out=xts[k % NB][:, :],
                            in_=x_d[r0:r1, t0 - 3 : t0 + T],
                        ).then_inc(din[k % NB], 16)

            @block.scalar
            def _(scalar):
                def finish(j):
                    # SiLU + output DMA for tile j (m[j] complete once
                    # dve >= j+1). The DMA rides ACT's own HWDGE ring and
                    # follows the activation in program order, which the
                    # sequencer only advances past on completion.
                    scalar.wait_ge(dve, j + 1)
                    if j >= NB:
                        # yt slot's previous store (tile j-NB) must be done
                        scalar.wait_ge(dout[j % NB], 16 * (j // NB))
                    func = getattr(AF, _ACT_FUNC)
                    scalar.activation(
                        out=yts[j % NB][:, :], in_=ms[j % NB][:, :],
                        func=func,
                        bias=0.0 if func == AF.Copy else zb,
                        scale=1.0,
                    ).then_inc(act)
                    # the DMA trigger races ahead of the still-streaming
                    # activation write; self-wait on its completion inc
                    scalar.wait_ge(act, min(j + 2, NTILES) + j + 1)
                    r0, r1 = x_rows(j)
                    t0 = (j % NT) * T
                    scalar.dma_start(
                        out=y_d[r0:r1, t0 : t0 + T], in_=yts[j % NB][:, :]
                    ).then_inc(dout[j % NB], 16)

                scalar.wait_ge(wsem, 16)
                for k in range(NTILES):
                    scalar.wait_ge(din[k % NB], 16 * (k // NB + 1))
                    xt = xts[k % NB]
                    m = ms[k % NB]
                    if k % NT == 0:
                        # causal start: begin with tap 3 over the full tile
                        scalar.activation(
                            out=m[:, :], in_=xt[:, 3 : 3 + T],
                            func=AF.Copy, bias=0.0, scale=w_ap(k, 3),
                        ).then_inc(act)
                    else:
                        scalar.activation(
                            out=m[:, :], in_=xt[:, 0:T],
                            func=AF.Copy, bias=0.0, scale=w_ap(k, 0),
                        ).then_inc(act)
                    if k >= 1:
                        finish(k - 1)
                finish(NTILES - 1)
                for i in range(NB):
                    n_stores = len([k for k in range(NTILES) if k % NB == i])
                    scalar.wait_ge(dout[i], 16 * n_stores)

            @block.vector
            def _(vector):
                def stt(out, in0, scalar_ap, in1):
                    return vector.scalar_tensor_tensor(
                        out=out, in0=in0, scalar=scalar_ap, in1=in1,
                        op0=mult, op1=add,
                    )

                for k in range(NTILES):
                    vector.wait_ge(act, 1 if k == 0 else 2 * k)
                    xt = xts[k % NB]
                    m = ms[k % NB]
                    if k % NT == 0:
                        # column j only sees taps with j-3+i >= 0
                        stt(m[:, 1:T], xt[:, 3 : T + 2], w_ap(k, 2), m[:, 1:T])
                        stt(m[:, 2:T], xt[:, 3 : T + 1], w_ap(k, 1), m[:, 2:T])
                        stt(m[:, 3:T], xt[:, 3:T], w_ap(k, 0), m[:, 3:T]).then_inc(dve)
                    else:
                        stt(m[:, :], xt[:, 1 : 1 + T], w_ap(k, 1), m[:, :])
                        stt(m[:, :], xt[:, 2 : 2 + T], w_ap(k, 2), m[:, :])
                        stt(m[:, :], xt[:, 3 : 3 + T], w_ap(k, 3), m[:, :]).then_inc(dve)

    return nc


def kernel(x, weight):
    global _last_results
    from concourse.bass_utils import run_bass_kernel_spmd

    x = np.asarray(x, dtype=np.float32)
    weight = np.asarray(weight, dtype=np.float32)

    nc = _build_program()

    in_maps = []
    for core in range(N_CORES):
        sl = slice(core * HC, (core + 1) * HC)
        # [B, S, HC] -> [B, HC, S] -> [ROWS, S], row r = b*HC + c
        xs = np.ascontiguousarray(x[:, :, sl].transpose(0, 2, 1)).reshape(ROWS, S)
        ws = weight[sl, :]  # (HC, K)
        w_host = np.zeros((128, NU * K + 1), np.float32)
        for u in range(NU):
            blk = u % (HC // 128)
            w_host[:, u * K : (u + 1) * K] = ws[blk * 128 : (blk + 1) * 128, :]
        in_maps.append({"x": xs, "w": w_host})

    res = run_bass_kernel_spmd(nc, in_maps, list(range(N_CORES)))
    _last_results = res

    out = np.empty((B, S, H), np.float32)
    for core in range(N_CORES):
        sl = slice(core * HC, (core + 1) * HC)
        yc = res.results[core]["y"].reshape(B, HC, S)
        out[:, :, sl] = yc.transpose(0, 2, 1)
    return out


# revision 19
# speedup vs baseline: 1.8769x; 1.1452x over previous
"""Causal depthwise conv1d (K=4) + SiLU, sharded over 8 NeuronCores.

Full shapes: x [4, 8192, 2048] f32, weight [2048, 4] f32 -> y [4, 8192, 2048] f32.

Strategy: tensor-parallel over the hidden/channel dim (fully channel
independent, no halo exchange). Each core gets 256 channels, reorganized
host-side to channel-major [B*256, S] so the conv runs along the free dim with
channels on SBUF partitions.

Compute: all 4 taps run on the TensorEngine as float32r diagonal-matrix
matmuls accumulating in PSUM (psum[c,t] += w_i[c] * x[c, t-3+i] via
diag(w_i) @ x_shifted). The causal left edge falls out of PSUM's per-element
has_written semantics: each tap's matmul covers only its valid columns, and
uncovered columns are overwritten-not-accumulated by the first tap that does
cover them. DVE only rounds fp32 -> fp32r (required by the fp32r matmul
contract); ACT applies SiLU straight out of PSUM and triggers the output DMA
on its own HWDGE ring.

Raw bass (no Tile framework): the installed walrus codegen only accepts one
sync wait per compute instruction, so all synchronization is explicit wait_ge
sequencer instructions. Per-buffer-slot DMA semaphores keep concurrent DMA
completion increments unambiguous. Sem increments fire at instruction
completion, but the sequencer runs ahead, so DMA triggers/consumers of an
engine's result always gate on that completion increment (including
same-engine self-waits before DMA triggers).
"""

import contextlib

import numpy as np

B, S, H, K = 4, 8192, 2048, 4
N_CORES = 8
HC = H // N_CORES          # 256 channels per core
ROWS = B * HC              # 1024 rows per core, row r = b*HC + c
NU = ROWS // 128           # 8 partition units
T = 2048                   # token tile
NT = S // T
NTILES = NU * NT           # 32
NB = 4                     # buffers per tile kind
NC_CHUNK = 512             # one PSUM bank of fp32
NCHUNKS = T // NC_CHUNK

_last_results = None       # test harness introspection (exec_time_ns etc.)
_ACT_FUNC = "Silu"         # sim override hook (CoreSim lacks Silu)


def _build_program():
    from concourse import bass, mybir

    f32 = mybir.dt.float32
    f32r = mybir.dt.float32r
    AF = mybir.ActivationFunctionType

    nc = bass.Bass()
    x_d = nc.declare_dram_parameter("x", [ROWS, S], f32, isOutput=False)
    # per-unit diagonal weight matrices [128 x 128] for each tap, + 1 zero col
    w_d = nc.declare_dram_parameter(
        "w", [128, NU * K * 128 + 1], f32, isOutput=False
    )
    y_d = nc.declare_dram_parameter("y", [ROWS, S], f32, isOutput=True)

    with contextlib.ExitStack() as st:
        wt = st.enter_context(nc.sbuf_tensor("wt", [128, NU * K * 128 + 1], f32))
        wtr = st.enter_context(nc.sbuf_tensor("wtr", [128, NU * K * 128], f32r))
        xts = [
            st.enter_context(nc.sbuf_tensor(f"xt{i}", [128, T + 3], f32))
            for i in range(NB)
        ]
        xrs = [
            st.enter_context(nc.sbuf_tensor(f"xr{i}", [128, T + 3], f32r))
            for i in range(NB)
        ]
        yts = [
            st.enter_context(nc.sbuf_tensor(f"yt{i}", [128, T], f32))
            for i in range(NB)
        ]
        pss = [
            st.enter_context(nc.psum_tensor(f"ps{i}", [128, T], f32))
            for i in range(2)
        ]
        zb = wt[:, NU * K * 128 : NU * K * 128 + 1]   # zeros column (Silu bias)

        def wdiag(k, i):
            u = k // NT
            c0 = (u * K + i) * 128
            return wtr[:, c0 : c0 + 128]

        def x_rows(k):
            r0 = (k // NT) * 128
            return r0, r0 + 128

        with (
            nc.Block() as block,
            nc.semaphore("wsem") as wsem,
            nc.semaphore("act") as act,
            nc.semaphore("dve") as dve,
            nc.semaphore("pe") as pe,
            contextlib.ExitStack() as sems,
        ):
            din = [
                sems.enter_context(nc.semaphore(f"din{i}")) for i in range(NB)
            ]
            dout = [
                sems.enter_context(nc.semaphore(f"dout{i}")) for i in range(NB)
            ]

            @block.sync
            def _(sync):
                sync.dma_start(out=wt[:, :], in_=w_d[:, :]).then_inc(wsem, 16)
                for k in range(NTILES):
                    r0, r1 = x_rows(k)
                    t = k % NT
                    t0 = t * T
                    if k >= NB:
                        # xt slot free once DVE rounded tile k-NB out of it
                        sync.wait_ge(dve, k - NB + 2)
                    if t == 0:
                        sync.dma_start(
                            out=xts[k % NB][:, 3 : T + 3],
                            in_=x_d[r0:r1, 0:T],
                        ).then_inc(din[k % NB], 16)
                    else:
                        sync.dma_start(
                            out=xts[k % NB][:, :],
                            in_=x_d[r0:r1, t0 - 3 : t0 + T],
                        ).then_inc(din[k % NB], 16)

            @block.vector
            def _(vector):
                # round the diag weights once (fp32r inputs must be rounded
                # by their producer)
                vector.wait_ge(wsem, 16)
                vector.tensor_copy(
                    out=wtr[:, :], in_=wt[:, 0 : NU * K * 128]
                ).then_inc(dve)
                for k in range(NTILES):
                    vector.wait_ge(din[k % NB], 16 * (k // NB + 1))
                    if k >= NB:
                        # xr slot free once PE consumed tile k-NB
                        vector.wait_ge(pe, k - NB + 1)
                    if k % NT == 0:
                        # causal left edge: zero the halo, then round as usual
                        vector.memset(xts[k % NB][:, 0:3], 0.0)
                    vector.tensor_copy(
                        out=xrs[k % NB][:, :], in_=xts[k % NB][:, :]
                    ).then_inc(dve)

            @block.tensor
            def _(tensor):
                for k in range(NTILES):
                    tensor.wait_ge(dve, k + 2)      # weights + round_k done
                    if k >= 2:
                        # psum buffer free once silu of tile k-2 done
                        tensor.wait_ge(act, k - 1)
                    ps = pss[k % 2]
                    xr = xrs[k % NB]
                    for c in range(NCHUNKS):
                        c0 = c * NC_CHUNK
                        for i in range(K):
                            mm = tensor.matmul(
                                ps[:, c0 : c0 + NC_CHUNK],
                                wdiag(k, i),
                                xr[:, c0 + i : c0 + i + NC_CHUNK],
                                start=(i == 0),
                                stop=(i == K - 1),
                                skip_group_check=True,
                            )
                    mm.then_inc(pe)

            @block.scalar
            def _(scalar):
                func = getattr(AF, _ACT_FUNC)
                for k in range(NTILES):
                    scalar.wait_ge(pe, k + 1)
                    if k >= NB:
                        # yt slot's previous store (tile k-NB) must be done
                        scalar.wait_ge(dout[k % NB], 16 * (k // NB))
                    scalar.activation(
                        out=yts[k % NB][:, :], in_=pss[k % 2][:, :],
                        func=func,
                        bias=0.0 if func == AF.Copy else zb,
                        scale=1.0,
                    ).then_inc(act)
                    # the DMA trigger races ahead of the still-streaming
                    # activation write; self-wait on its completion inc
                    scalar.wait_ge(act, k + 1)
                    r0, r1 = x_rows(k)
                    t0 = (k % NT) * T
                    scalar.dma_start(
                        out=y_d[r0:r1, t0 : t0 + T], in_=yts[k % NB][:, :]
                    ).then_inc(dout[k % NB], 16)
                for i in range(NB):
                    n_stores = len([k for k in range(NTILES) if k % NB == i])
                    scalar.wait_ge(dout[i], 16 * n_stores)

    return nc


def kernel(x, weight):
    global _last_results
    from concourse.bass_utils import run_bass_kernel_spmd

    x = np.asarray(x, dtype=np.float32)
    weight = np.asarray(weight, dtype=np.float32)

    nc = _build_program()

    in_maps = []
    eye = np.eye(128, dtype=np.float32)
    for core in range(N_CORES):
        sl = slice(core * HC, (core + 1) * HC)
        # [B, S, HC] -> [B, HC, S] -> [ROWS, S], row r = b*HC + c
        xs = np.ascontiguousarray(x[:, :, sl].transpose(0, 2, 1)).reshape(ROWS, S)
        ws = weight[sl, :]  # (HC, K)
        w_host = np.zeros((128, NU * K * 128 + 1), np.float32)
        for u in range(NU):
            blk = u % (HC // 128)
            for i in range(K):
                c0 = (u * K + i) * 128
                w_host[:, c0 : c0 + 128] = eye * ws[blk * 128 : (blk + 1) * 128, i]
        in_maps.append({"x": xs, "w": w_host})

    res = run_bass_kernel_spmd(nc, in_maps, list(range(N_CORES)))
    _last_results = res

    out = np.empty((B, S, H), np.float32)
    for core in range(N_CORES):
        sl = slice(core * HC, (core + 1) * HC)
        yc = res.results[core]["y"].reshape(B, HC, S)
        out[:, :, sl] = yc.transpose(0, 2, 1)
    return out
